# revision 36
# baseline (speedup 1.0000x reference)
"""Distributed Bass attention-head kernel for one TRN2 chip (8 NeuronCores).

Problem: x[8192,1024], Wk/Wq/Wv[64,1024] ->
  out  = softmax((x Wq^T)(x Wk^T)^T / sqrt(64)) @ (x Wv^T)   [8192, 64]
  obj  = pre-softmax affinity row 1                            [1, 8192]

Design notes (measured on silicon):
- ncfw's first collective per execution has a ~50us doorbell->mesh-begin
  spin-up, so gathered data cannot exist before ~70us. K^T is therefore
  REPLICATED: every core computes the full K^T from the full x^T (streamed,
  ~16MB bf16) so the scores+exp pipeline starts at ~10us instead.
- V' (V with a ones column, so the PV matmul also produces the softmax
  denominator) is the one remaining all-gather; PV consumes it late, after
  the exp stream has banked ~48 chunks of results in SBUF.
- Each core computes out rows for its own 1024 q positions:
  S^T[kv=128, q=1024] = K^T_chunk^T @ q^T  (scale folded into Wq),
  P = exp(S^T) with no max-subtraction (|scores| <~ 8 in fp32),
  acc[65, q] += V'_chunk^T @ P, epilogue transposes acc and divides.
"""

import os

import numpy as np
import ml_dtypes

import concourse.bass as bass
import concourse.tile as tile
from concourse import bacc, mybir
from concourse.bass import ts, ds
from concourse.bass_utils import run_bass_kernel_spmd
from concourse.masks import make_identity

T, C, H = 8192, 1024, 64
NCORES = 8
TLOC = T // NCORES            # 1024 rows of x per core
SCALE = H ** -0.5
NKV = T // 128                # 64 kv chunks of 128
NCC = C // 128                # 8 contraction chunks of 128
NT = TLOC // 128              # 8 local 128-row tiles
VLEN = TLOC * (H + 1)         # elems of local v' [1024, 65]
ES_BUFS = 64

BF = mybir.dt.bfloat16
F32 = mybir.dt.float32

LAST_EXEC_TIME_NS = None
_CACHED = {}


def _install_profile_hook():
    """Make trace=True work on the bare axon agent image.

    concourse's axon trace path reads the NTFF hook via
    ``antenv.axon_hooks``; on this image that module is absent, so
    synthesize it and register the ctypes-based hook from trn_boot.
    """
    import sys
    import types

    try:
        from antenv.axon_hooks import get_axon_ntff_profile_hook  # noqa: F401

        return True
    except ImportError:
        pass
    try:
        import antenv
        from trn_agent_boot.trn_boot import _ntff_profile_via_ctypes

        so_path = "/opt/axon/libaxon_pjrt.so"
        if not os.path.exists(so_path):
            return False
        hook = _ntff_profile_via_ctypes(so_path)
        if hook is None:
            return False
        mod = types.ModuleType("antenv.axon_hooks")
        mod._hook = hook
        mod.get_axon_ntff_profile_hook = lambda: mod._hook
        mod.set_axon_ntff_profile_hook = lambda h: setattr(mod, "_hook", h)
        sys.modules["antenv.axon_hooks"] = mod
        antenv.axon_hooks = mod
        return True
    except Exception:
        return False


def build_nc():
    nc = bacc.Bacc(None, debug=False, num_devices=NCORES)

    xT = nc.declare_dram_parameter("xT", [C, TLOC], BF, isOutput=False)
    xTf = nc.declare_dram_parameter("xTf", [C, T], BF, isOutput=False)
    wq = nc.declare_dram_parameter("wqT", [C, H], BF, isOutput=False)
    wk = nc.declare_dram_parameter("wkT", [C, H], BF, isOutput=False)
    wv = nc.declare_dram_parameter("wvT", [C, H], BF, isOutput=False)
    out_e = nc.declare_dram_parameter("out", [TLOC, H], F32, isOutput=True)
    obj_e = nc.declare_dram_parameter("obj", [128, NKV], F32, isOutput=True)

    ccv_in = nc.dram_tensor("ccv_in", [VLEN], BF)
    ccv_out = nc.dram_tensor("ccv_out", [NCORES * VLEN], BF, addr_space="Shared")

    with tile.TileContext(nc) as tc:
        with (
            tc.tile_pool(name="const", bufs=1) as constp,
            tc.tile_pool(name="xf", bufs=2) as xfpool,
            tc.tile_pool(name="wts", bufs=1) as wpool,
            tc.tile_pool(name="proj", bufs=1) as projpool,
            tc.tile_pool(name="big", bufs=1) as bigpool,
            tc.tile_pool(name="es", bufs=ES_BUFS) as espool,
            tc.tile_pool(name="ep", bufs=2) as eppool,
            tc.tile_pool(name="ps_s", bufs=3, space="PSUM") as ps_s,
            tc.tile_pool(name="ps_acc", bufs=1, space="PSUM") as ps_acc,
        ):
            # weights, then the local x^T shard
            wk_sb = wpool.tile([128, NCC * H], BF, name="wk_sb")
            wv_sb = wpool.tile([128, NCC * H], BF, name="wv_sb")
            wq_sb = wpool.tile([128, NCC * H], BF, name="wq_sb")
            for w_sb, w_ext in ((wv_sb, wv), (wq_sb, wq), (wk_sb, wk)):
                nc.sync.dma_start(
                    w_sb.rearrange("p (c h) -> p c h", c=NCC),
                    w_ext.rearrange("(c p) h -> p c h", p=128),
                )
            # local x^T shard into an xfg-pool slot (freed after the q/v
            # projections so the full-x stream can reuse it)
            xl = xfpool.tile([128, NCC * TLOC], BF, name="xl", tag="xfg")
            nc.sync.dma_start(
                xl.rearrange("p (c t) -> p c t", c=NCC),
                xT.rearrange("(c p) t -> p c t", p=128),
            )

            ident_b = constp.tile([64, 64], BF, name="ident_b")
            make_identity(nc, ident_b)
            ident_f = constp.tile([H + 1, H + 1], F32, name="ident_f")
            make_identity(nc, ident_f)

            def project(w_sb, dst):
                for j in range(TLOC // 512):
                    pp = ps_s.tile([64, 512], F32, name="pp", tag="pss")
                    for c in range(NCC):
                        nc.tensor.matmul(
                            pp[:, :],
                            lhsT=w_sb[:, ds(c * H, H)],
                            rhs=xl[:, ds(c * TLOC + j * 512, 512)],
                            start=(c == 0),
                            stop=(c == NCC - 1),
                        )
                    nc.vector.tensor_copy(dst[:, ts(j, 512)], pp[:, :])

            # q^T first (scores depend on it), duplicated to partitions
            # 64-127 so scores for odd kv chunks can run as concurrent
            # row-group-64 matmuls
            qT2 = projpool.tile([128, TLOC], BF, name="qT2")
            project(wq_sb, qT2[0:64, :])
            nc.sync.dma_start(qT2[64:128, :], qT2[0:64, :])

            # v': local v^T -> transpose -> ones column -> all-gather.
            # The collective's ~50-70us ncfw spin-up (anchored at NEFF
            # launch) runs while K^T/scores proceed.  The staging DMA goes
            # on the scalar HWDGE queue so it doesn't stall the sync-queue
            # x^T stream behind the vpall dependency.
            vT = projpool.tile([64, TLOC], BF, name="vT")
            project(wv_sb, vT)
            vpall = constp.tile([128, NT * (H + 1)], BF, name="vpall")
            for t in range(NT):
                pv = ps_s.tile([128, 64], BF, name="pv", tag="pss")
                nc.tensor.transpose(pv[:, :], vT[:, ts(t, 128)], ident_b[:, :])
                nc.vector.tensor_copy(vpall[:, ds(t * (H + 1), H)], pv[:, :])
                nc.vector.memset(vpall[:, ds(t * (H + 1) + H, 1)], 1.0)
            nc.scalar.dma_start(
                ccv_in.rearrange("(tt p h) -> p tt h", p=128, h=H + 1),
                vpall.rearrange("p (tt h) -> p tt h", tt=NT),
            )
            nc.gpsimd.collective_compute(
                "AllGather",
                mybir.AluOpType.bypass,
                replica_groups=[list(range(NCORES))],
                ins=[ccv_in[:]],
                outs=[ccv_out[:]],
            )

            # replicated K^T, partition-packed: even kv chunks on
            # partitions 0-63, odd on 64-127; pair pi at cols [pi*128,+128)
            kfull = bigpool.tile([128, T // 2], BF, name="kfull")
            vg = bigpool.tile([128, NKV * (H + 1)], BF, name="vg")
            obj_sb = constp.tile([128, NKV], F32, name="obj_sb")
            po = [
                ps_acc.tile([H + 1, 512], F32, name=f"po{j}", tag=f"po{j}")
                for j in range(2)
            ]
            es_tiles = []

            def pv(ci):
                for j in range(2):
                    nc.tensor.matmul(
                        po[j][:, :],
                        lhsT=vg[:, ds(ci * (H + 1), H + 1)],
                        rhs=es_tiles[ci][:, ts(j, 512)],
                        start=(ci == 0),
                        stop=(ci == NKV - 1),
                    )

            for g in range(NCORES):
                xfg = xfpool.tile([128, NCC * TLOC], BF, name="xfg", tag="xfg")
                nc.sync.dma_start(
                    xfg.rearrange("p (c t) -> p c t", c=NCC),
                    xTf.rearrange("(c p) t -> p c t", p=128)[:, :, ts(g, TLOC)],
                )
                for jj in range(2):
                    # t-range [g*1024+jj*512, +512) = kv chunks 4jj..4jj+3 of
                    # this g; even chunks land on psum partitions 0-63, odd
                    # on 64-127 (concurrent col-group-64 matmuls)
                    pk = ps_s.tile([128, 512], F32, name="pk", tag="pss")
                    for c in range(NCC):
                        xslab = xfg[:, ds(c * TLOC + jj * 512, 512)].rearrange(
                            "p (b2 b1 t) -> p b2 b1 t", b2=2, b1=2
                        )
                        nc.tensor.matmul(
                            pk[0:64, 0:256],
                            lhsT=wk_sb[:, ds(c * H, H)],
                            rhs=xslab[:, :, 0, :],
                            start=(c == 0),
                            stop=(c == NCC - 1),
                        )
                        nc.tensor.matmul(
                            pk[64:128, 0:256],
                            lhsT=wk_sb[:, ds(c * H, H)],
                            rhs=xslab[:, :, 1, :],
                            start=(c == 0),
                            stop=(c == NCC - 1),
                        )
                    nc.vector.tensor_copy(
                        kfull[:, ds(g * 512 + jj * 256, 256)], pk[:, 0:256]
                    )
                for tt in range(NT // 2):
                    pi = g * (NT // 2) + tt
                    pss_eo = []
                    for eo in range(2):
                        lo, hi = 64 * eo, 64 * eo + 64
                        pss = ps_s.tile([128, TLOC], F32, name="pss", tag="pss")
                        for j in range(2):
                            nc.tensor.matmul(
                                pss[:, ts(j, 512)],
                                lhsT=kfull[lo:hi, ts(pi, 128)],
                                rhs=qT2[lo:hi, ts(j, 512)],
                                start=True,
                                stop=True,
                            )
                        pss_eo.append(pss)
                    for eo in range(2):
                        ci = 2 * pi + eo
                        pss = pss_eo[eo]
                        es = espool.tile([128, TLOC], BF, name="es", tag="es")
                        nc.scalar.activation(
                            es[:, :], pss[:, :],
                            mybir.ActivationFunctionType.Exp,
                        )
                        # affinity row q=1, pre-softmax (column 1 of chunk)
                        nc.vector.tensor_copy(
                            obj_sb[:, ds(ci, 1)], pss[:, ds(1, 1)]
                        )
                        es_tiles.append(es)

                if g == 6:
                    # The collective's real mesh cannot begin before ~60-80us
                    # after launch (ncfw init) — the cost model doesn't know
                    # that, so pin the V'-consuming work late or the
                    # scheduler interleaves it into the PE stream and the
                    # in-order PE queue stalls on it.  PV of chunks 0..15
                    # goes here so their es slots free up for chunks 48..63.
                    with tc.tile_wait_until(1.0):
                        vgv = vg.rearrange("p (ci h) -> p ci h", h=H + 1)
                        ccvv = ccv_out.rearrange(
                            "(ci p h) -> p ci h", p=128, h=H + 1
                        )
                        for b in range(4):
                            nc.sync.dma_start(
                                vgv[:, ts(b, 16)], ccvv[:, ts(b, 16)]
                            )
                        for ci in range(NKV - ES_BUFS):
                            pv(ci)

            # PV accumulation tail (consumes banked es tiles)
            with tc.tile_wait_until(1.2):
                for ci in range(NKV - ES_BUFS, NKV):
                    pv(ci)

            nc.sync.dma_start(obj_e[:, :], obj_sb[:, :])

            # epilogue: transpose acc back to [q, 65], divide by denominator
            for j in range(2):
                oT = eppool.tile([H + 1, 512], F32, name="oT", tag="oT")
                nc.vector.tensor_copy(oT[:, :], po[j][:, :])
                for s in range(4):
                    pt = ps_s.tile([128, H + 1], F32, name="pt", tag="pss")
                    nc.tensor.transpose(pt[:, :], oT[:, ts(s, 128)], ident_f[:, :])
                    ot = eppool.tile([128, H + 1], F32, name="ot", tag="ot")
                    nc.scalar.copy(ot[:, :], pt[:, :])
                    rec = eppool.tile([128, 1], F32, name="rec", tag="rec")
                    nc.vector.reciprocal(rec[:, :], ot[:, ds(H, 1)])
                    res = eppool.tile([128, H], F32, name="res", tag="res")
                    nc.vector.tensor_scalar_mul(res[:, :], ot[:, 0:H], rec[:, :])
                    nc.sync.dma_start(
                        out_e[ds(j * 512 + s * 128, 128), :], res[:, :]
                    )

    nc.compile()
    return nc


def kernel(x, Wk, Wq, Wv):
    global LAST_EXEC_TIME_NS
    x = np.asarray(x, dtype=np.float32)
    Wk = np.asarray(Wk, dtype=np.float32)
    Wq = np.asarray(Wq, dtype=np.float32)
    Wv = np.asarray(Wv, dtype=np.float32)

    bf = ml_dtypes.bfloat16
    xTb = np.ascontiguousarray(x.T).astype(bf)             # [C, T]
    wqb = np.ascontiguousarray((Wq * SCALE).T).astype(bf)  # [C, H], scale folded
    wkb = np.ascontiguousarray(Wk.T).astype(bf)
    wvb = np.ascontiguousarray(Wv.T).astype(bf)

    if "nc" not in _CACHED:
        _CACHED["nc"] = build_nc()
    nc = _CACHED["nc"]

    in_maps = [
        {
            "xT": np.ascontiguousarray(xTb[:, g * TLOC : (g + 1) * TLOC]),
            "xTf": xTb,
            "wqT": wqb,
            "wkT": wkb,
            "wvT": wvb,
        }
        for g in range(NCORES)
    ]

    trace = os.environ.get("KERNEL_TRACE", "1") == "1"
    if trace:
        trace = _install_profile_hook()
    r = None
    if trace:
        try:
            r = run_bass_kernel_spmd(
                nc, in_maps, core_ids=list(range(NCORES)), trace=True
            )
        except Exception as e:
            print(f"traced run failed ({e!r}); retrying untraced")
            r = None
    if r is None:
        r = run_bass_kernel_spmd(
            nc, in_maps, core_ids=list(range(NCORES)), trace=False
        )
    LAST_EXEC_TIME_NS = r.exec_time_ns
    results = r.results

    out_full = np.concatenate(
        [np.asarray(results[g]["out"], dtype=np.float32) for g in range(NCORES)],
        axis=0,
    )
    # obj buffer: [p, chunk] with affinity row 1 at kv = chunk*128 + p
    objbuf = np.asarray(results[0]["obj"], dtype=np.float32)
    obj_full = np.ascontiguousarray(objbuf.T.reshape(1, T))
    return out_full, obj_full


# revision 37
# speedup vs baseline: 1.1184x; 1.1184x over previous
"""Distributed Bass attention-head kernel for one TRN2 chip (8 NeuronCores).

Problem: x[8192,1024], Wk/Wq/Wv[64,1024] ->
  out  = softmax((x Wq^T)(x Wk^T)^T / sqrt(64)) @ (x Wv^T)   [8192, 64]
  obj  = pre-softmax affinity row 1                            [1, 8192]

Design notes (measured on silicon):
- ncfw's first collective per execution has a ~50us doorbell->mesh-begin
  spin-up, so gathered data cannot exist before ~70us. K^T is therefore
  REPLICATED: every core computes the full K^T from the full x^T (streamed,
  ~16MB bf16) so the scores+exp pipeline starts at ~10us instead.
- V' (V with a ones column, so the PV matmul also produces the softmax
  denominator) is the one remaining all-gather; PV consumes it late, after
  the exp stream has banked ~48 chunks of results in SBUF.
- Each core computes out rows for its own 1024 q positions:
  S^T[kv=128, q=1024] = K^T_chunk^T @ q^T  (scale folded into Wq),
  P = exp(S^T) with no max-subtraction (|scores| <~ 8 in fp32),
  acc[65, q] += V'_chunk^T @ P, epilogue transposes acc and divides.
"""

import os

import numpy as np
import ml_dtypes

import concourse.bass as bass
import concourse.tile as tile
from concourse import bacc, mybir
from concourse.bass import ts, ds
from concourse.bass_utils import run_bass_kernel_spmd
from concourse.masks import make_identity

T, C, H = 8192, 1024, 64
NCORES = 8
TLOC = T // NCORES            # 1024 rows of x per core
SCALE = H ** -0.5
NKV = T // 128                # 64 kv chunks of 128
NCC = C // 128                # 8 contraction chunks of 128
NT = TLOC // 128              # 8 local 128-row tiles
VLEN = TLOC * (H + 1)         # elems of local v' [1024, 65]
ES_BUFS = 56

BF = mybir.dt.bfloat16
F32 = mybir.dt.float32

LAST_EXEC_TIME_NS = None
_CACHED = {}


def _install_profile_hook():
    """Make trace=True work on the bare axon agent image.

    concourse's axon trace path reads the NTFF hook via
    ``antenv.axon_hooks``; on this image that module is absent, so
    synthesize it and register the ctypes-based hook from trn_boot.
    """
    import sys
    import types

    try:
        from antenv.axon_hooks import get_axon_ntff_profile_hook  # noqa: F401

        return True
    except ImportError:
        pass
    try:
        import antenv
        from trn_agent_boot.trn_boot import _ntff_profile_via_ctypes

        so_path = "/opt/axon/libaxon_pjrt.so"
        if not os.path.exists(so_path):
            return False
        hook = _ntff_profile_via_ctypes(so_path)
        if hook is None:
            return False
        mod = types.ModuleType("antenv.axon_hooks")
        mod._hook = hook
        mod.get_axon_ntff_profile_hook = lambda: mod._hook
        mod.set_axon_ntff_profile_hook = lambda h: setattr(mod, "_hook", h)
        sys.modules["antenv.axon_hooks"] = mod
        antenv.axon_hooks = mod
        return True
    except Exception:
        return False


def build_nc():
    nc = bacc.Bacc(None, debug=False, num_devices=NCORES)

    xT = nc.declare_dram_parameter("xT", [C, TLOC], BF, isOutput=False)
    xTf = nc.declare_dram_parameter("xTf", [C, T], BF, isOutput=False)
    wq = nc.declare_dram_parameter("wqT", [C, H], BF, isOutput=False)
    wk = nc.declare_dram_parameter("wkT", [C, H], BF, isOutput=False)
    wv = nc.declare_dram_parameter("wvT", [C, H], BF, isOutput=False)
    out_e = nc.declare_dram_parameter("out", [TLOC, H], F32, isOutput=True)
    obj_e = nc.declare_dram_parameter("obj", [128, NKV], F32, isOutput=True)

    ccv_in = nc.dram_tensor("ccv_in", [VLEN], BF)
    ccv_out = nc.dram_tensor("ccv_out", [NCORES * VLEN], BF, addr_space="Shared")

    with tile.TileContext(nc) as tc:
        with (
            tc.tile_pool(name="const", bufs=1) as constp,
            tc.tile_pool(name="xts", bufs=NCC) as xpool,
            tc.tile_pool(name="xf", bufs=2) as xfpool,
            tc.tile_pool(name="wts", bufs=1) as wpool,
            tc.tile_pool(name="proj", bufs=1) as projpool,
            tc.tile_pool(name="big", bufs=1) as bigpool,
            tc.tile_pool(name="es", bufs=ES_BUFS) as espool,
            tc.tile_pool(name="ep", bufs=2) as eppool,
            tc.tile_pool(name="ps_s", bufs=3, space="PSUM") as ps_s,
            tc.tile_pool(name="ps_acc", bufs=1, space="PSUM") as ps_acc,
        ):
            # weights, then the local x^T shard
            wk_sb = wpool.tile([128, NCC * H], BF, name="wk_sb")
            wv_sb = wpool.tile([128, NCC * H], BF, name="wv_sb")
            wq_sb = wpool.tile([128, NCC * H], BF, name="wq_sb")
            for w_sb, w_ext in ((wv_sb, wv), (wq_sb, wq), (wk_sb, wk)):
                nc.sync.dma_start(
                    w_sb.rearrange("p (c h) -> p c h", c=NCC),
                    w_ext.rearrange("(c p) h -> p c h", p=128),
                )
            xts = []
            for c in range(NCC):
                xt = xpool.tile([128, TLOC], BF, name=f"xt{c}", tag="xt")
                nc.sync.dma_start(xt[:, :], xT[ts(c, 128), :])
                xts.append(xt)

            ident_b = constp.tile([64, 64], BF, name="ident_b")
            make_identity(nc, ident_b)
            ident_f = constp.tile([H + 1, H + 1], F32, name="ident_f")
            make_identity(nc, ident_f)

            def project(w_sb, dst):
                for j in range(TLOC // 512):
                    pp = ps_s.tile([64, 512], F32, name="pp", tag="pss")
                    for c in range(NCC):
                        nc.tensor.matmul(
                            pp[:, :],
                            lhsT=w_sb[:, ds(c * H, H)],
                            rhs=xts[c][:, ts(j, 512)],
                            start=(c == 0),
                            stop=(c == NCC - 1),
                        )
                    nc.vector.tensor_copy(dst[:, ts(j, 512)], pp[:, :])

            # q^T first (scores depend on it), duplicated to partitions
            # 64-127 so scores for odd kv chunks can run as concurrent
            # row-group-64 matmuls
            qT2 = projpool.tile([128, TLOC], BF, name="qT2")
            project(wq_sb, qT2[0:64, :])
            nc.sync.dma_start(qT2[64:128, :], qT2[0:64, :])

            # v': local v^T -> transpose -> ones column -> all-gather.
            # The collective's ~50-70us ncfw spin-up (anchored at NEFF
            # launch) runs while K^T/scores proceed.  The staging DMA goes
            # on the scalar HWDGE queue so it doesn't stall the sync-queue
            # x^T stream behind the vpall dependency.
            vT = projpool.tile([64, TLOC], BF, name="vT")
            project(wv_sb, vT)
            vpall = constp.tile([128, NT * (H + 1)], BF, name="vpall")
            for t in range(NT):
                pv = ps_s.tile([128, 64], BF, name="pv", tag="pss")
                nc.tensor.transpose(pv[:, :], vT[:, ts(t, 128)], ident_b[:, :])
                nc.vector.tensor_copy(vpall[:, ds(t * (H + 1), H)], pv[:, :])
                nc.vector.memset(vpall[:, ds(t * (H + 1) + H, 1)], 1.0)
            nc.scalar.dma_start(
                ccv_in.rearrange("(tt p h) -> p tt h", p=128, h=H + 1),
                vpall.rearrange("p (tt h) -> p tt h", tt=NT),
            )
            nc.gpsimd.collective_compute(
                "AllGather",
                mybir.AluOpType.bypass,
                replica_groups=[list(range(NCORES))],
                ins=[ccv_in[:]],
                outs=[ccv_out[:]],
            )

            # replicated K^T, partition-packed: even kv chunks on
            # partitions 0-63, odd on 64-127; pair pi at cols [pi*128,+128)
            kfull = bigpool.tile([128, T // 2], BF, name="kfull")
            vg = bigpool.tile([128, NKV * (H + 1)], BF, name="vg")
            obj_sb = constp.tile([128, NKV], F32, name="obj_sb")
            po = [
                ps_acc.tile([H + 1, 512], F32, name=f"po{j}", tag=f"po{j}")
                for j in range(2)
            ]
            es_tiles = []

            def pv(ci):
                for j in range(2):
                    nc.tensor.matmul(
                        po[j][:, :],
                        lhsT=vg[:, ds(ci * (H + 1), H + 1)],
                        rhs=es_tiles[ci][:, ts(j, 512)],
                        start=(ci == 0),
                        stop=(ci == NKV - 1),
                    )

            for g in range(NCORES):
                xfg = xfpool.tile([128, NCC * TLOC], BF, name="xfg", tag="xfg")
                nc.sync.dma_start(
                    xfg.rearrange("p (c t) -> p c t", c=NCC),
                    xTf.rearrange("(c p) t -> p c t", p=128)[:, :, ts(g, TLOC)],
                )
                for jj in range(2):
                    # t-range [g*1024+jj*512, +512) = kv chunks 4jj..4jj+3 of
                    # this g; even chunks land on psum partitions 0-63, odd
                    # on 64-127 (concurrent col-group-64 matmuls)
                    pk = ps_s.tile([128, 512], F32, name="pk", tag="pss")
                    for c in range(NCC):
                        xslab = xfg[:, ds(c * TLOC + jj * 512, 512)].rearrange(
                            "p (b2 b1 t) -> p b2 b1 t", b2=2, b1=2
                        )
                        nc.tensor.matmul(
                            pk[0:64, 0:256],
                            lhsT=wk_sb[:, ds(c * H, H)],
                            rhs=xslab[:, :, 0, :],
                            start=(c == 0),
                            stop=(c == NCC - 1),
                        )
                        nc.tensor.matmul(
                            pk[64:128, 0:256],
                            lhsT=wk_sb[:, ds(c * H, H)],
                            rhs=xslab[:, :, 1, :],
                            start=(c == 0),
                            stop=(c == NCC - 1),
                        )
                    nc.vector.tensor_copy(
                        kfull[:, ds(g * 512 + jj * 256, 256)], pk[:, 0:256]
                    )
                for tt in range(NT // 2):
                    pi = g * (NT // 2) + tt
                    pss_eo = []
                    for eo in range(2):
                        lo, hi = 64 * eo, 64 * eo + 64
                        pss = ps_s.tile([128, TLOC], F32, name="pss", tag="pss")
                        for j in range(2):
                            nc.tensor.matmul(
                                pss[:, ts(j, 512)],
                                lhsT=kfull[lo:hi, ts(pi, 128)],
                                rhs=qT2[lo:hi, ts(j, 512)],
                                start=True,
                                stop=True,
                            )
                        pss_eo.append(pss)
                    for eo in range(2):
                        ci = 2 * pi + eo
                        pss = pss_eo[eo]
                        es = espool.tile([128, TLOC], BF, name="es", tag="es")
                        nc.scalar.activation(
                            es[:, :], pss[:, :],
                            mybir.ActivationFunctionType.Exp,
                        )
                        # affinity row q=1, pre-softmax (column 1 of chunk)
                        nc.vector.tensor_copy(
                            obj_sb[:, ds(ci, 1)], pss[:, ds(1, 1)]
                        )
                        es_tiles.append(es)

                if g == 6:
                    # The collective's real mesh cannot begin before ~60-80us
                    # after launch (ncfw init) — the cost model doesn't know
                    # that, so pin the V'-consuming work late or the
                    # scheduler interleaves it into the PE stream and the
                    # in-order PE queue stalls on it.  PV of chunks 0..15
                    # goes here so their es slots free up for chunks 48..63.
                    with tc.tile_wait_until(1.0):
                        vgv = vg.rearrange("p (ci h) -> p ci h", h=H + 1)
                        ccvv = ccv_out.rearrange(
                            "(ci p h) -> p ci h", p=128, h=H + 1
                        )
                        for b in range(4):
                            nc.sync.dma_start(
                                vgv[:, ts(b, 16)], ccvv[:, ts(b, 16)]
                            )
                        for ci in range(NKV - ES_BUFS):
                            pv(ci)

            # PV accumulation tail (consumes banked es tiles)
            with tc.tile_wait_until(1.2):
                for ci in range(NKV - ES_BUFS, NKV):
                    pv(ci)

            nc.sync.dma_start(obj_e[:, :], obj_sb[:, :])

            # epilogue: transpose acc back to [q, 65], divide by denominator
            for j in range(2):
                oT = eppool.tile([H + 1, 512], F32, name="oT", tag="oT")
                nc.vector.tensor_copy(oT[:, :], po[j][:, :])
                for s in range(4):
                    pt = ps_s.tile([128, H + 1], F32, name="pt", tag="pss")
                    nc.tensor.transpose(pt[:, :], oT[:, ts(s, 128)], ident_f[:, :])
                    ot = eppool.tile([128, H + 1], F32, name="ot", tag="ot")
                    nc.scalar.copy(ot[:, :], pt[:, :])
                    rec = eppool.tile([128, 1], F32, name="rec", tag="rec")
                    nc.vector.reciprocal(rec[:, :], ot[:, ds(H, 1)])
                    res = eppool.tile([128, H], F32, name="res", tag="res")
                    nc.vector.tensor_scalar_mul(res[:, :], ot[:, 0:H], rec[:, :])
                    nc.sync.dma_start(
                        out_e[ds(j * 512 + s * 128, 128), :], res[:, :]
                    )

    nc.compile()
    return nc


def kernel(x, Wk, Wq, Wv):
    global LAST_EXEC_TIME_NS
    x = np.asarray(x, dtype=np.float32)
    Wk = np.asarray(Wk, dtype=np.float32)
    Wq = np.asarray(Wq, dtype=np.float32)
    Wv = np.asarray(Wv, dtype=np.float32)

    bf = ml_dtypes.bfloat16
    xTb = np.ascontiguousarray(x.T).astype(bf)             # [C, T]
    wqb = np.ascontiguousarray((Wq * SCALE).T).astype(bf)  # [C, H], scale folded
    wkb = np.ascontiguousarray(Wk.T).astype(bf)
    wvb = np.ascontiguousarray(Wv.T).astype(bf)

    if "nc" not in _CACHED:
        _CACHED["nc"] = build_nc()
    nc = _CACHED["nc"]

    in_maps = [
        {
            "xT": np.ascontiguousarray(xTb[:, g * TLOC : (g + 1) * TLOC]),
            "xTf": xTb,
            "wqT": wqb,
            "wkT": wkb,
            "wvT": wvb,
        }
        for g in range(NCORES)
    ]

    trace = os.environ.get("KERNEL_TRACE", "1") == "1"
    if trace:
        trace = _install_profile_hook()
    r = None
    if trace:
        try:
            r = run_bass_kernel_spmd(
                nc, in_maps, core_ids=list(range(NCORES)), trace=True
            )
        except Exception as e:
            print(f"traced run failed ({e!r}); retrying untraced")
            r = None
    if r is None:
        r = run_bass_kernel_spmd(
            nc, in_maps, core_ids=list(range(NCORES)), trace=False
        )
    LAST_EXEC_TIME_NS = r.exec_time_ns
    results = r.results

    out_full = np.concatenate(
        [np.asarray(results[g]["out"], dtype=np.float32) for g in range(NCORES)],
        axis=0,
    )
    # obj buffer: [p, chunk] with affinity row 1 at kv = chunk*128 + p
    objbuf = np.asarray(results[0]["obj"], dtype=np.float32)
    obj_full = np.ascontiguousarray(objbuf.T.reshape(1, T))
    return out_full, obj_full


# revision 40
# speedup vs baseline: 1.1390x; 1.0183x over previous
"""Distributed Bass attention-head kernel for one TRN2 chip (8 NeuronCores).

Problem: x[8192,1024], Wk/Wq/Wv[64,1024] ->
  out  = softmax((x Wq^T)(x Wk^T)^T / sqrt(64)) @ (x Wv^T)   [8192, 64]
  obj  = pre-softmax affinity row 1                            [1, 8192]

Design notes (measured on silicon):
- ncfw's first collective per execution has a ~50us doorbell->mesh-begin
  spin-up, so gathered data cannot exist before ~70us. K^T is therefore
  REPLICATED: every core computes the full K^T from the full x^T (streamed,
  ~16MB bf16) so the scores+exp pipeline starts at ~10us instead.
- V' (V with a ones column, so the PV matmul also produces the softmax
  denominator) is the one remaining all-gather; PV consumes it late, after
  the exp stream has banked ~48 chunks of results in SBUF.
- Each core computes out rows for its own 1024 q positions:
  S^T[kv=128, q=1024] = K^T_chunk^T @ q^T  (scale folded into Wq),
  P = exp(S^T) with no max-subtraction (|scores| <~ 8 in fp32),
  acc[65, q] += V'_chunk^T @ P, epilogue transposes acc and divides.
"""

import os

import numpy as np
import ml_dtypes

import concourse.bass as bass
import concourse.tile as tile
from concourse import bacc, mybir
from concourse.bass import ts, ds
from concourse.bass_utils import run_bass_kernel_spmd
from concourse.masks import make_identity

T, C, H = 8192, 1024, 64
NCORES = 8
TLOC = T // NCORES            # 1024 rows of x per core
SCALE = H ** -0.5
NKV = T // 128                # 64 kv chunks of 128
NCC = C // 128                # 8 contraction chunks of 128
NT = TLOC // 128              # 8 local 128-row tiles
VLEN = TLOC * (H + 1)         # elems of local v' [1024, 65]
ES_BUFS = 56

BF = mybir.dt.bfloat16
F32 = mybir.dt.float32

LAST_EXEC_TIME_NS = None
_CACHED = {}


def _install_profile_hook():
    """Make trace=True work on the bare axon agent image.

    concourse's axon trace path reads the NTFF hook via
    ``antenv.axon_hooks``; on this image that module is absent, so
    synthesize it and register the ctypes-based hook from trn_boot.
    """
    import sys
    import types

    try:
        from antenv.axon_hooks import get_axon_ntff_profile_hook  # noqa: F401

        return True
    except ImportError:
        pass
    try:
        import antenv
        from trn_agent_boot.trn_boot import _ntff_profile_via_ctypes

        so_path = "/opt/axon/libaxon_pjrt.so"
        if not os.path.exists(so_path):
            return False
        hook = _ntff_profile_via_ctypes(so_path)
        if hook is None:
            return False
        mod = types.ModuleType("antenv.axon_hooks")
        mod._hook = hook
        mod.get_axon_ntff_profile_hook = lambda: mod._hook
        mod.set_axon_ntff_profile_hook = lambda h: setattr(mod, "_hook", h)
        sys.modules["antenv.axon_hooks"] = mod
        antenv.axon_hooks = mod
        return True
    except Exception:
        return False


def build_nc():
    nc = bacc.Bacc(None, debug=False, num_devices=NCORES)

    xT = nc.declare_dram_parameter("xT", [C, TLOC], BF, isOutput=False)
    xTf = nc.declare_dram_parameter("xTf", [C, T], BF, isOutput=False)
    wq = nc.declare_dram_parameter("wqT", [C, H], BF, isOutput=False)
    wk = nc.declare_dram_parameter("wkT", [C, H], BF, isOutput=False)
    wv = nc.declare_dram_parameter("wvT", [C, H], BF, isOutput=False)
    out_e = nc.declare_dram_parameter("out", [TLOC, H], F32, isOutput=True)
    obj_e = nc.declare_dram_parameter("obj", [128, NKV], F32, isOutput=True)

    HVLEN = VLEN // 2
    ccva_in = nc.dram_tensor("ccva_in", [HVLEN], BF)
    ccva_out = nc.dram_tensor("ccva_out", [NCORES * HVLEN], BF, addr_space="Shared")
    ccvb_in = nc.dram_tensor("ccvb_in", [HVLEN], BF)
    ccvb_out = nc.dram_tensor("ccvb_out", [NCORES * HVLEN], BF, addr_space="Shared")

    with tile.TileContext(nc) as tc:
        with (
            tc.tile_pool(name="const", bufs=1) as constp,
            tc.tile_pool(name="xts", bufs=NCC) as xpool,
            tc.tile_pool(name="xf", bufs=2) as xfpool,
            tc.tile_pool(name="wts", bufs=1) as wpool,
            tc.tile_pool(name="proj", bufs=1) as projpool,
            tc.tile_pool(name="big", bufs=1) as bigpool,
            tc.tile_pool(name="es", bufs=ES_BUFS) as espool,
            tc.tile_pool(name="ep", bufs=2) as eppool,
            tc.tile_pool(name="ps_s", bufs=3, space="PSUM") as ps_s,
            tc.tile_pool(name="ps_acc", bufs=1, space="PSUM") as ps_acc,
        ):
            # weights, then the local x^T shard
            wk_sb = wpool.tile([128, NCC * H], BF, name="wk_sb")
            wv_sb = wpool.tile([128, NCC * H], BF, name="wv_sb")
            wq_sb = wpool.tile([128, NCC * H], BF, name="wq_sb")
            for w_sb, w_ext in ((wv_sb, wv), (wq_sb, wq), (wk_sb, wk)):
                nc.sync.dma_start(
                    w_sb.rearrange("p (c h) -> p c h", c=NCC),
                    w_ext.rearrange("(c p) h -> p c h", p=128),
                )
            xts = []
            for c in range(NCC):
                xt = xpool.tile([128, TLOC], BF, name=f"xt{c}", tag="xt")
                nc.sync.dma_start(xt[:, :], xT[ts(c, 128), :])
                xts.append(xt)

            ident_b = constp.tile([64, 64], BF, name="ident_b")
            make_identity(nc, ident_b)
            ident_f = constp.tile([H + 1, H + 1], F32, name="ident_f")
            make_identity(nc, ident_f)

            def project(w_sb, dst):
                for j in range(TLOC // 512):
                    pp = ps_s.tile([64, 512], F32, name="pp", tag="pss")
                    for c in range(NCC):
                        nc.tensor.matmul(
                            pp[:, :],
                            lhsT=w_sb[:, ds(c * H, H)],
                            rhs=xts[c][:, ts(j, 512)],
                            start=(c == 0),
                            stop=(c == NCC - 1),
                        )
                    nc.vector.tensor_copy(dst[:, ts(j, 512)], pp[:, :])

            # q^T first (scores depend on it), duplicated to partitions
            # 64-127 so scores for odd kv chunks can run as concurrent
            # row-group-64 matmuls
            qT2 = projpool.tile([128, TLOC], BF, name="qT2")
            project(wq_sb, qT2[0:64, :])
            nc.scalar.dma_start(qT2[64:128, :], qT2[0:64, :])

            # v': local v^T -> transpose -> ones column -> all-gather.
            # The collective's ~50-70us ncfw spin-up (anchored at NEFF
            # launch) runs while K^T/scores proceed.  The staging DMA goes
            # on the scalar HWDGE queue so it doesn't stall the sync-queue
            # x^T stream behind the vpall dependency.
            vT = projpool.tile([64, TLOC], BF, name="vT")
            project(wv_sb, vT)
            vpall = constp.tile([128, NT * (H + 1)], BF, name="vpall")
            for half, cin, cout in ((0, ccva_in, ccva_out), (1, ccvb_in, ccvb_out)):
                for t in range(half * 4, half * 4 + 4):
                    pv = ps_s.tile([128, 64], BF, name="pv", tag="pss")
                    nc.tensor.transpose(
                        pv[:, :], vT[:, ts(t, 128)], ident_b[:, :]
                    )
                    nc.vector.tensor_copy(
                        vpall[:, ds(t * (H + 1), H)], pv[:, :]
                    )
                    nc.vector.memset(vpall[:, ds(t * (H + 1) + H, 1)], 1.0)
                nc.scalar.dma_start(
                    cin.rearrange("(tt p h) -> p tt h", p=128, h=H + 1),
                    vpall[:, ds(half * 4 * (H + 1), 4 * (H + 1))].rearrange(
                        "p (tt h) -> p tt h", tt=4
                    ),
                )
                nc.gpsimd.collective_compute(
                    "AllGather",
                    mybir.AluOpType.bypass,
                    replica_groups=[list(range(NCORES))],
                    ins=[cin[:]],
                    outs=[cout[:]],
                )

            # replicated K^T, partition-packed: even kv chunks on
            # partitions 0-63, odd on 64-127; pair pi at cols [pi*128,+128)
            kfull = bigpool.tile([128, T // 2], BF, name="kfull")
            vg = bigpool.tile([128, NKV * (H + 1)], BF, name="vg")
            obj_sb = constp.tile([128, NKV], F32, name="obj_sb")
            po = [
                ps_acc.tile([H + 1, 512], F32, name=f"po{j}", tag=f"po{j}")
                for j in range(2)
            ]
            es_tiles = []

            def pv(ci):
                for j in range(2):
                    nc.tensor.matmul(
                        po[j][:, :],
                        lhsT=vg[:, ds(ci * (H + 1), H + 1)],
                        rhs=es_tiles[ci][:, ts(j, 512)],
                        start=(ci == 0),
                        stop=(ci == NKV - 1),
                    )

            for g in range(NCORES):
                xfg = xfpool.tile([128, NCC * TLOC], BF, name="xfg", tag="xfg")
                nc.sync.dma_start(
                    xfg.rearrange("p (c t) -> p c t", c=NCC),
                    xTf.rearrange("(c p) t -> p c t", p=128)[:, :, ts(g, TLOC)],
                )
                for jj in range(2):
                    # t-range [g*1024+jj*512, +512) = kv chunks 4jj..4jj+3 of
                    # this g; even chunks land on psum partitions 0-63, odd
                    # on 64-127 (concurrent col-group-64 matmuls)
                    pk = ps_s.tile([128, 512], F32, name="pk", tag="pss")
                    for c in range(NCC):
                        xslab = xfg[:, ds(c * TLOC + jj * 512, 512)].rearrange(
                            "p (b2 b1 t) -> p b2 b1 t", b2=2, b1=2
                        )
                        nc.tensor.matmul(
                            pk[0:64, 0:256],
                            lhsT=wk_sb[:, ds(c * H, H)],
                            rhs=xslab[:, :, 0, :],
                            start=(c == 0),
                            stop=(c == NCC - 1),
                        )
                        nc.tensor.matmul(
                            pk[64:128, 0:256],
                            lhsT=wk_sb[:, ds(c * H, H)],
                            rhs=xslab[:, :, 1, :],
                            start=(c == 0),
                            stop=(c == NCC - 1),
                        )
                    nc.vector.tensor_copy(
                        kfull[:, ds(g * 512 + jj * 256, 256)], pk[:, 0:256]
                    )
                for tt in range(NT // 2):
                    pi = g * (NT // 2) + tt
                    pss_eo = []
                    for eo in range(2):
                        lo, hi = 64 * eo, 64 * eo + 64
                        pss = ps_s.tile([128, TLOC], F32, name="pss", tag="pss")
                        for j in range(2):
                            nc.tensor.matmul(
                                pss[:, ts(j, 512)],
                                lhsT=kfull[lo:hi, ts(pi, 128)],
                                rhs=qT2[lo:hi, ts(j, 512)],
                                start=True,
                                stop=True,
                            )
                        pss_eo.append(pss)
                    for eo in range(2):
                        ci = 2 * pi + eo
                        pss = pss_eo[eo]
                        es = espool.tile([128, TLOC], BF, name="es", tag="es")
                        nc.scalar.activation(
                            es[:, :], pss[:, :],
                            mybir.ActivationFunctionType.Exp,
                        )
                        # affinity row q=1, pre-softmax (column 1 of chunk)
                        nc.vector.tensor_copy(
                            obj_sb[:, ds(ci, 1)], pss[:, ds(1, 1)]
                        )
                        es_tiles.append(es)

                if g == 6:
                    # The collective's real mesh cannot begin before ~60-80us
                    # after launch (ncfw init) — the cost model doesn't know
                    # that, so pin the V'-consuming work late or the
                    # scheduler interleaves it into the PE stream and the
                    # in-order PE queue stalls on it.  PV of chunks 0..15
                    # goes here so their es slots free up for chunks 48..63.
                    with tc.tile_wait_until(1.0):
                        vsrca = ccva_out.rearrange(
                            "(g tt p h) -> g p tt h",
                            g=NCORES, tt=4, p=128,
                        )
                        for gg in range(NCORES):
                            nc.sync.dma_start(
                                vg[:, ds(gg * NT * (H + 1), 4 * (H + 1))]
                                .rearrange("p (tt h) -> p tt h", tt=4),
                                vsrca[gg],
                            )
                        for ci in range(ES_BUFS):
                            if ci % NT < 4:
                                pv(ci)

            # PV accumulation tail: second V' half + remaining chunks
            with tc.tile_wait_until(1.2):
                vsrcb = ccvb_out.rearrange(
                    "(g tt p h) -> g p tt h",
                    g=NCORES, tt=4, p=128,
                )
                for gg in range(NCORES):
                    nc.sync.dma_start(
                        vg[:, ds(gg * NT * (H + 1) + 4 * (H + 1), 4 * (H + 1))]
                        .rearrange("p (tt h) -> p tt h", tt=4),
                        vsrcb[gg],
                    )
                for ci in range(NKV):
                    if ci % NT >= 4 or ci >= ES_BUFS:
                        pv(ci)

            nc.sync.dma_start(obj_e[:, :], obj_sb[:, :])

            # epilogue: transpose acc back to [q, 65], divide by denominator
            for j in range(2):
                oT = eppool.tile([H + 1, 512], F32, name="oT", tag="oT")
                nc.vector.tensor_copy(oT[:, :], po[j][:, :])
                for s in range(4):
                    pt = ps_s.tile([128, H + 1], F32, name="pt", tag="pss")
                    nc.tensor.transpose(pt[:, :], oT[:, ts(s, 128)], ident_f[:, :])
                    ot = eppool.tile([128, H + 1], F32, name="ot", tag="ot")
                    nc.scalar.copy(ot[:, :], pt[:, :])
                    rec = eppool.tile([128, 1], F32, name="rec", tag="rec")
                    nc.vector.reciprocal(rec[:, :], ot[:, ds(H, 1)])
                    res = eppool.tile([128, H], F32, name="res", tag="res")
                    nc.vector.tensor_scalar_mul(res[:, :], ot[:, 0:H], rec[:, :])
                    nc.sync.dma_start(
                        out_e[ds(j * 512 + s * 128, 128), :], res[:, :]
                    )

    nc.compile()
    return nc


def kernel(x, Wk, Wq, Wv):
    global LAST_EXEC_TIME_NS
    x = np.asarray(x, dtype=np.float32)
    Wk = np.asarray(Wk, dtype=np.float32)
    Wq = np.asarray(Wq, dtype=np.float32)
    Wv = np.asarray(Wv, dtype=np.float32)

    bf = ml_dtypes.bfloat16
    xTb = np.ascontiguousarray(x.T).astype(bf)             # [C, T]
    wqb = np.ascontiguousarray((Wq * SCALE).T).astype(bf)  # [C, H], scale folded
    wkb = np.ascontiguousarray(Wk.T).astype(bf)
    wvb = np.ascontiguousarray(Wv.T).astype(bf)

    if "nc" not in _CACHED:
        _CACHED["nc"] = build_nc()
    nc = _CACHED["nc"]

    in_maps = [
        {
            "xT": np.ascontiguousarray(xTb[:, g * TLOC : (g + 1) * TLOC]),
            "xTf": xTb,
            "wqT": wqb,
            "wkT": wkb,
            "wvT": wvb,
        }
        for g in range(NCORES)
    ]

    trace = os.environ.get("KERNEL_TRACE", "1") == "1"
    if trace:
        trace = _install_profile_hook()
    r = None
    if trace:
        try:
            r = run_bass_kernel_spmd(
                nc, in_maps, core_ids=list(range(NCORES)), trace=True
            )
        except Exception as e:
            print(f"traced run failed ({e!r}); retrying untraced")
            r = None
    if r is None:
        r = run_bass_kernel_spmd(
            nc, in_maps, core_ids=list(range(NCORES)), trace=False
        )
    LAST_EXEC_TIME_NS = r.exec_time_ns
    results = r.results

    out_full = np.concatenate(
        [np.asarray(results[g]["out"], dtype=np.float32) for g in range(NCORES)],
        axis=0,
    )
    # obj buffer: [p, chunk] with affinity row 1 at kv = chunk*128 + p
    objbuf = np.asarray(results[0]["obj"], dtype=np.float32)
    obj_full = np.ascontiguousarray(objbuf.T.reshape(1, T))
    return out_full, obj_full


# revision 43
# speedup vs baseline: 1.1490x; 1.0088x over previous
"""Distributed Bass attention-head kernel for one TRN2 chip (8 NeuronCores).

Problem: x[8192,1024], Wk/Wq/Wv[64,1024] ->
  out  = softmax((x Wq^T)(x Wk^T)^T / sqrt(64)) @ (x Wv^T)   [8192, 64]
  obj  = pre-softmax affinity row 1                            [1, 8192]

Design notes (measured on silicon):
- ncfw's first collective per execution has a ~50us doorbell->mesh-begin
  spin-up, so gathered data cannot exist before ~70us. K^T is therefore
  REPLICATED: every core computes the full K^T from the full x^T (streamed,
  ~16MB bf16) so the scores+exp pipeline starts at ~10us instead.
- V' (V with a ones column, so the PV matmul also produces the softmax
  denominator) is the one remaining all-gather; PV consumes it late, after
  the exp stream has banked ~48 chunks of results in SBUF.
- Each core computes out rows for its own 1024 q positions:
  S^T[kv=128, q=1024] = K^T_chunk^T @ q^T  (scale folded into Wq),
  P = exp(S^T) with no max-subtraction (|scores| <~ 8 in fp32),
  acc[65, q] += V'_chunk^T @ P, epilogue transposes acc and divides.
"""

import os

import numpy as np
import ml_dtypes

import concourse.bass as bass
import concourse.tile as tile
from concourse import bacc, mybir
from concourse.bass import ts, ds
from concourse.bass_utils import run_bass_kernel_spmd
from concourse.masks import make_identity

T, C, H = 8192, 1024, 64
NCORES = 8
TLOC = T // NCORES            # 1024 rows of x per core
SCALE = H ** -0.5
NKV = T // 128                # 64 kv chunks of 128
NCC = C // 128                # 8 contraction chunks of 128
NT = TLOC // 128              # 8 local 128-row tiles
VLEN = TLOC * (H + 1)         # elems of local v' [1024, 65]
ES_BUFS = 56

BF = mybir.dt.bfloat16
F32 = mybir.dt.float32

LAST_EXEC_TIME_NS = None
_CACHED = {}


def _install_profile_hook():
    """Make trace=True work on the bare axon agent image.

    concourse's axon trace path reads the NTFF hook via
    ``antenv.axon_hooks``; on this image that module is absent, so
    synthesize it and register the ctypes-based hook from trn_boot.
    """
    import sys
    import types

    try:
        from antenv.axon_hooks import get_axon_ntff_profile_hook  # noqa: F401

        return True
    except ImportError:
        pass
    try:
        import antenv
        from trn_agent_boot.trn_boot import _ntff_profile_via_ctypes

        so_path = "/opt/axon/libaxon_pjrt.so"
        if not os.path.exists(so_path):
            return False
        hook = _ntff_profile_via_ctypes(so_path)
        if hook is None:
            return False
        mod = types.ModuleType("antenv.axon_hooks")
        mod._hook = hook
        mod.get_axon_ntff_profile_hook = lambda: mod._hook
        mod.set_axon_ntff_profile_hook = lambda h: setattr(mod, "_hook", h)
        sys.modules["antenv.axon_hooks"] = mod
        antenv.axon_hooks = mod
        return True
    except Exception:
        return False


def build_nc():
    nc = bacc.Bacc(None, debug=False, num_devices=NCORES)

    xT = nc.declare_dram_parameter("xT", [C, TLOC], BF, isOutput=False)
    xTf = nc.declare_dram_parameter("xTf", [C, T], BF, isOutput=False)
    wq = nc.declare_dram_parameter("wqT", [C, H], BF, isOutput=False)
    wk = nc.declare_dram_parameter("wkT", [C, H], BF, isOutput=False)
    wv = nc.declare_dram_parameter("wvT", [C, H], BF, isOutput=False)
    out_e = nc.declare_dram_parameter("out", [TLOC, H], F32, isOutput=True)
    obj_e = nc.declare_dram_parameter("obj", [128, NKV], F32, isOutput=True)

    ccv_in = nc.dram_tensor("ccv_in", [VLEN], BF)
    ccv_out = nc.dram_tensor("ccv_out", [NCORES * VLEN], BF, addr_space="Shared")

    with tile.TileContext(nc) as tc:
        with (
            tc.tile_pool(name="const", bufs=1) as constp,
            tc.tile_pool(name="xts", bufs=NCC) as xpool,
            tc.tile_pool(name="xf", bufs=2) as xfpool,
            tc.tile_pool(name="wts", bufs=1) as wpool,
            tc.tile_pool(name="proj", bufs=1) as projpool,
            tc.tile_pool(name="big", bufs=1) as bigpool,
            tc.tile_pool(name="es", bufs=ES_BUFS) as espool,
            tc.tile_pool(name="ep", bufs=2) as eppool,
            tc.tile_pool(name="ps_s", bufs=3, space="PSUM") as ps_s,
            tc.tile_pool(name="ps_acc", bufs=1, space="PSUM") as ps_acc,
        ):
            # weights, then the local x^T shard
            wk_sb = wpool.tile([128, NCC * H], BF, name="wk_sb")
            wv_sb = wpool.tile([128, NCC * H], BF, name="wv_sb")
            wq_sb = wpool.tile([128, NCC * H], BF, name="wq_sb")
            for w_sb, w_ext in ((wv_sb, wv), (wq_sb, wq), (wk_sb, wk)):
                nc.sync.dma_start(
                    w_sb.rearrange("p (c h) -> p c h", c=NCC),
                    w_ext.rearrange("(c p) h -> p c h", p=128),
                )
            xts = []
            for c in range(NCC):
                xt = xpool.tile([128, TLOC], BF, name=f"xt{c}", tag="xt")
                nc.sync.dma_start(xt[:, :], xT[ts(c, 128), :])
                xts.append(xt)

            ident_b = constp.tile([64, 64], BF, name="ident_b")
            make_identity(nc, ident_b)
            ident_f = constp.tile([H + 1, H + 1], F32, name="ident_f")
            make_identity(nc, ident_f)

            def project(w_sb, dst):
                for j in range(TLOC // 512):
                    pp = ps_s.tile([64, 512], F32, name="pp", tag="pss")
                    for c in range(NCC):
                        nc.tensor.matmul(
                            pp[:, :],
                            lhsT=w_sb[:, ds(c * H, H)],
                            rhs=xts[c][:, ts(j, 512)],
                            start=(c == 0),
                            stop=(c == NCC - 1),
                        )
                    nc.vector.tensor_copy(dst[:, ts(j, 512)], pp[:, :])

            # q^T first (scores depend on it), duplicated to partitions
            # 64-127 so scores for odd kv chunks can run as concurrent
            # row-group-64 matmuls
            qT2 = projpool.tile([128, TLOC], BF, name="qT2")
            project(wq_sb, qT2[0:64, :])
            nc.sync.dma_start(qT2[64:128, :], qT2[0:64, :])

            # v': local v^T -> transpose -> ones column -> all-gather.
            # The collective's ~50-70us ncfw spin-up (anchored at NEFF
            # launch) runs while K^T/scores proceed.  The staging DMA goes
            # on the scalar HWDGE queue so it doesn't stall the sync-queue
            # x^T stream behind the vpall dependency.
            vT = projpool.tile([64, TLOC], BF, name="vT")
            project(wv_sb, vT)
            vpall = constp.tile([128, NT * (H + 1)], BF, name="vpall")
            for t in range(NT):
                pv = ps_s.tile([128, 64], BF, name="pv", tag="pss")
                nc.tensor.transpose(pv[:, :], vT[:, ts(t, 128)], ident_b[:, :])
                nc.vector.tensor_copy(vpall[:, ds(t * (H + 1), H)], pv[:, :])
                nc.vector.memset(vpall[:, ds(t * (H + 1) + H, 1)], 1.0)
            nc.scalar.dma_start(
                ccv_in.rearrange("(tt p h) -> p tt h", p=128, h=H + 1),
                vpall.rearrange("p (tt h) -> p tt h", tt=NT),
            )
            nc.gpsimd.collective_compute(
                "AllGather",
                mybir.AluOpType.bypass,
                replica_groups=[list(range(NCORES))],
                ins=[ccv_in[:]],
                outs=[ccv_out[:]],
            )

            # replicated K^T, partition-packed: even kv chunks on
            # partitions 0-63, odd on 64-127; pair pi at cols [pi*128,+128)
            kfull = bigpool.tile([128, T // 2], BF, name="kfull")
            vg = bigpool.tile([128, NKV * (H + 1)], BF, name="vg")
            obj_sb = constp.tile([128, NKV], F32, name="obj_sb")
            po = [
                ps_acc.tile([H + 1, 512], F32, name=f"po{j}", tag=f"po{j}")
                for j in range(2)
            ]
            es_tiles = []

            def pv(ci):
                for j in range(2):
                    nc.tensor.matmul(
                        po[j][:, :],
                        lhsT=vg[:, ds(ci * (H + 1), H + 1)],
                        rhs=es_tiles[ci][:, ts(j, 512)],
                        start=(ci == 0),
                        stop=(ci == NKV - 1),
                    )

            for g in range(NCORES):
                xfg = xfpool.tile([128, NCC * TLOC], BF, name="xfg", tag="xfg")
                nc.sync.dma_start(
                    xfg.rearrange("p (c t) -> p c t", c=NCC),
                    xTf.rearrange("(c p) t -> p c t", p=128)[:, :, ts(g, TLOC)],
                )
                for jj in range(2):
                    # t-range [g*1024+jj*512, +512) = kv chunks 4jj..4jj+3 of
                    # this g; even chunks land on psum partitions 0-63, odd
                    # on 64-127 (concurrent col-group-64 matmuls)
                    pk = ps_s.tile([128, 512], F32, name="pk", tag="pss")
                    for c in range(NCC):
                        xslab = xfg[:, ds(c * TLOC + jj * 512, 512)].rearrange(
                            "p (b2 b1 t) -> p b2 b1 t", b2=2, b1=2
                        )
                        nc.tensor.matmul(
                            pk[0:64, 0:256],
                            lhsT=wk_sb[:, ds(c * H, H)],
                            rhs=xslab[:, :, 0, :],
                            start=(c == 0),
                            stop=(c == NCC - 1),
                        )
                        nc.tensor.matmul(
                            pk[64:128, 0:256],
                            lhsT=wk_sb[:, ds(c * H, H)],
                            rhs=xslab[:, :, 1, :],
                            start=(c == 0),
                            stop=(c == NCC - 1),
                        )
                    nc.vector.tensor_copy(
                        kfull[:, ds(g * 512 + jj * 256, 256)], pk[:, 0:256]
                    )
                for tt in range(NT // 2):
                    pi = g * (NT // 2) + tt
                    pss_eo = []
                    for eo in range(2):
                        lo, hi = 64 * eo, 64 * eo + 64
                        pss = ps_s.tile([128, TLOC], F32, name="pss", tag="pss")
                        for j in range(2):
                            nc.tensor.matmul(
                                pss[:, ts(j, 512)],
                                lhsT=kfull[lo:hi, ts(pi, 128)],
                                rhs=qT2[lo:hi, ts(j, 512)],
                                start=True,
                                stop=True,
                            )
                        pss_eo.append(pss)
                    for eo in range(2):
                        ci = 2 * pi + eo
                        pss = pss_eo[eo]
                        es = espool.tile([128, TLOC], BF, name="es", tag="es")
                        nc.scalar.activation(
                            es[:, :], pss[:, :],
                            mybir.ActivationFunctionType.Exp,
                        )
                        # affinity row q=1, pre-softmax (column 1 of chunk)
                        nc.vector.tensor_copy(
                            obj_sb[:, ds(ci, 1)], pss[:, ds(1, 1)]
                        )
                        es_tiles.append(es)

                if g == 6:
                    # The collective's real mesh cannot begin before ~60-80us
                    # after launch (ncfw init) — the cost model doesn't know
                    # that, so pin the V'-consuming work late or the
                    # scheduler interleaves it into the PE stream and the
                    # in-order PE queue stalls on it.  PV of chunks 0..15
                    # goes here so their es slots free up for chunks 48..63.
                    with tc.tile_wait_until(1.0):
                        vgv = vg.rearrange("p (ci h) -> p ci h", h=H + 1)
                        ccvv = ccv_out.rearrange(
                            "(ci p h) -> p ci h", p=128, h=H + 1
                        )
                        for b in range(4):
                            nc.sync.dma_start(
                                vgv[:, ts(b, 16)], ccvv[:, ts(b, 16)]
                            )
                        for ci in range(NKV - ES_BUFS):
                            pv(ci)

            # PV accumulation tail (consumes banked es tiles)
            with tc.tile_wait_until(1.2):
                for ci in range(NKV - ES_BUFS, NKV):
                    pv(ci)

            nc.sync.dma_start(obj_e[:, :], obj_sb[:, :])

            # epilogue: transpose acc back to [q, 65], divide by denominator
            for j in range(2):
                oT = eppool.tile([H + 1, 512], F32, name="oT", tag="oT")
                nc.vector.tensor_copy(oT[:, :], po[j][:, :])
                for s in range(4):
                    pt = ps_s.tile([128, H + 1], F32, name="pt", tag="pss")
                    nc.tensor.transpose(pt[:, :], oT[:, ts(s, 128)], ident_f[:, :])
                    ot = eppool.tile([128, H + 1], F32, name="ot", tag="ot")
                    nc.scalar.copy(ot[:, :], pt[:, :])
                    rec = eppool.tile([128, 1], F32, name="rec", tag="rec")
                    nc.vector.reciprocal(rec[:, :], ot[:, ds(H, 1)])
                    res = eppool.tile([128, H], F32, name="res", tag="res")
                    nc.vector.tensor_scalar_mul(res[:, :], ot[:, 0:H], rec[:, :])
                    nc.sync.dma_start(
                        out_e[ds(j * 512 + s * 128, 128), :], res[:, :]
                    )

    nc.compile()
    return nc


def kernel(x, Wk, Wq, Wv):
    global LAST_EXEC_TIME_NS
    x = np.asarray(x, dtype=np.float32)
    Wk = np.asarray(Wk, dtype=np.float32)
    Wq = np.asarray(Wq, dtype=np.float32)
    Wv = np.asarray(Wv, dtype=np.float32)

    bf = ml_dtypes.bfloat16
    xTb = np.ascontiguousarray(x.T).astype(bf)             # [C, T]
    wqb = np.ascontiguousarray((Wq * SCALE).T).astype(bf)  # [C, H], scale folded
    wkb = np.ascontiguousarray(Wk.T).astype(bf)
    wvb = np.ascontiguousarray(Wv.T).astype(bf)

    if "nc" not in _CACHED:
        _CACHED["nc"] = build_nc()
    nc = _CACHED["nc"]

    in_maps = [
        {
            "xT": np.ascontiguousarray(xTb[:, g * TLOC : (g + 1) * TLOC]),
            "xTf": xTb,
            "wqT": wqb,
            "wkT": wkb,
            "wvT": wvb,
        }
        for g in range(NCORES)
    ]

    trace = os.environ.get("KERNEL_TRACE", "1") == "1"
    if trace:
        trace = _install_profile_hook()
    r = None
    if trace:
        try:
            r = run_bass_kernel_spmd(
                nc, in_maps, core_ids=list(range(NCORES)), trace=True
            )
        except Exception as e:
            print(f"traced run failed ({e!r}); retrying untraced")
            r = None
    if r is None:
        r = run_bass_kernel_spmd(
            nc, in_maps, core_ids=list(range(NCORES)), trace=False
        )
    LAST_EXEC_TIME_NS = r.exec_time_ns
    results = r.results

    out_full = np.concatenate(
        [np.asarray(results[g]["out"], dtype=np.float32) for g in range(NCORES)],
        axis=0,
    )
    # obj buffer: [p, chunk] with affinity row 1 at kv = chunk*128 + p
    objbuf = np.asarray(results[0]["obj"], dtype=np.float32)
    obj_full = np.ascontiguousarray(objbuf.T.reshape(1, T))
    return out_full, obj_full
